# revision 17
# baseline (speedup 1.0000x reference)
"""End-to-end memory network kernel for Trainium2 (8 NeuronCores, batch-sharded).

Per core (BC = B/8 = 256 batch items):
  1. indirect-DMA gather of embedding rows (the memory-roofline term).
     Host pre-arranges token indices in partition-major blocks so each
     gather call delivers [128 sentences x 32 tokens x 128 dim].
  2. TensorE identity-matmul accumulation reduces the 32 tokens of each
     sentence into a PSUM tile  ctx[sigma, D]  (sigma = b*50 + s).
  3. PE transpose -> ctxT [D, sigma], fused pos_enc add on the way to SBUF.
  4. 3 memory hops on ctxT in transposed layouts:
       pqT = W_h @ qT (+b) ;  scores via DVE mul + ones-matmul partition
       reduce ;  softmax on [b, S] tiles ;  retrieved via attn broadcast
       (ones-matmul) + DVE mul + segmented reduce ;  qT += retrievedT.
  5. logitsT = out_W @ qT + out_b.
Outputs logitsT [50, BC] and attn [3, BC, 50] per core; host reassembles.
"""

import os

import numpy as np

import concourse.bass as bass
import concourse.mybir as mybir
import concourse.tile as tile
from concourse import library_config
from concourse.bass_utils import run_bass_kernel_spmd
from concourse.masks import make_identity

# Problem dims (hardcoded per harness contract)
B, S, L = 2048, 50, 32
Q = 32
D = 128
HOPS = 3
VOCAB = 50000
NCORES = 8
BC = B // NCORES  # 256

P = 128
REBASE = 25000  # dma_gather index rebase: idx16 = v - REBASE, base = emb[REBASE:]
GTOK = 4096     # tokens per dma_gather call (= one 128-sentence block)
CHUNK_B = 64  # batch items per hop chunk; CHUNK_B*S = 3200 = 25 blocks of 128
BLK_PER_CHUNK = (CHUNK_B * S) // P  # 25
NCOLS = CHUNK_B * S  # 3200

f32 = mybir.dt.float32
i32 = mybir.dt.int32
bf16 = mybir.dt.bfloat16
EMB_BF16 = bool(int(os.environ.get("KERNEL_BF16", "0")))
EMB_DT = bf16 if EMB_BF16 else f32


def _split_multi_waits(nc: bass.Bass) -> None:
    """This walrus build encodes a single sem-wait slot on several ISA
    structs (LDWEIGHTS, pseudo-DMA, ...). Rewrite every instruction that
    carries >1 wait: move all but the last wait onto NoOps inserted just
    before it on the same engine stream."""
    for bb in nc.bb_map.values():
        insts = bb.bb.instructions
        out = []
        changed = False
        for inst in insts:
            si = inst.sync_info
            waits = list(si.on_wait) if si is not None and si.on_wait else []
            if len(waits) > 1:
                for w in waits[:-1]:
                    out.append(
                        mybir.InstNoOp(
                            name=nc.get_next_instruction_name(),
                            engine=inst.engine,
                            ins=[],
                            outs=[],
                            sync_info=mybir.SyncInfo(on_wait=[w], on_update=[]),
                        )
                    )
                si.on_wait = [waits[-1]]
                changed = True
            out.append(inst)
        if changed:
            insts[:] = out


def build_nc(bc: int = BC, split_waits: bool = True, rebase: int = REBASE) -> bass.Bass:
    nsig = bc * S
    nblk = nsig // P
    nchunk = bc // CHUNK_B
    nqblk = (bc * Q) // (P * Q // Q)  # bc questions / 128 per block
    nqblk = bc // P  # blocks of 128 questions
    assert nsig % P == 0 and bc % CHUNK_B == 0 and bc % P == 0

    nc = bass.Bass()

    emb_c = nc.dram_tensor("emb_c", [VOCAB, D], EMB_DT, kind="ExternalInput")
    emb_q = nc.dram_tensor("emb_q", [VOCAB, D], EMB_DT, kind="ExternalInput")
    i16 = mybir.dt.int16
    ctx_idx_d = nc.dram_tensor(
        "ctx_idx", [nblk, P, GTOK // 16], i16, kind="ExternalInput"
    )
    q_idx_d = nc.dram_tensor(
        "q_idx", [nqblk, P, GTOK // 16], i16, kind="ExternalInput"
    )
    posT_d = nc.dram_tensor("posT_rep", [P, NCOLS], f32, kind="ExternalInput")
    wT_d = nc.dram_tensor("wT", [HOPS, D, D], f32, kind="ExternalInput")
    hop_bT_d = nc.dram_tensor("hop_bT", [D, HOPS], f32, kind="ExternalInput")
    out_WT_d = nc.dram_tensor("out_WT", [D, S], f32, kind="ExternalInput")
    out_b_d = nc.dram_tensor("out_b_col", [S, 1], f32, kind="ExternalInput")

    debug = bool(int(os.environ.get("KERNEL_DEBUG", "0")))
    if debug:
        dbg_q0_d = nc.dram_tensor("debug_q0", [D, bc], f32, kind="ExternalOutput")
        dbg_ctxT0_d = nc.dram_tensor(
            "debug_ctxT0", [P, NCOLS], f32, kind="ExternalOutput"
        )
    logitsT_d = nc.dram_tensor("logitsT", [S, bc], f32, kind="ExternalOutput")
    attn_d = nc.dram_tensor("attn_out", [HOPS, bc, S], f32, kind="ExternalOutput")

    nc.gpsimd.load_library(library_config.mlp)

    with tile.TileContext(nc) as tc:
        with (
            tc.tile_pool(name="const", bufs=1) as cp,
            tc.tile_pool(name="big", bufs=1) as bp,
            tc.tile_pool(name="tok", bufs=2) as tkp,
            tc.tile_pool(name="stage", bufs=3) as stp,
            tc.tile_pool(name="hop", bufs=2) as hp,
            tc.tile_pool(name="q", bufs=2 * HOPS + 2) as qp,
            tc.tile_pool(name="psA", bufs=2, space="PSUM") as psA,
            tc.tile_pool(name="psB", bufs=2, space="PSUM") as psB,
            tc.tile_pool(name="psW", bufs=3, space="PSUM") as psW,
            tc.tile_pool(name="psS", bufs=1, space="PSUM") as psS,
        ):
            # ---- constants / params ----
            ident = cp.tile([P, P], f32)
            make_identity(nc, ident[:])
            if EMB_BF16:
                ident_e = cp.tile([P, P], bf16, name="ident_e")
                nc.vector.tensor_copy(out=ident_e[:], in_=ident[:])
            else:
                ident_e = ident
            ones_col = cp.tile([P, 1], f32)
            nc.vector.memset(ones_col[:], 1.0)
            ones_row = cp.tile([1, P], f32)
            nc.vector.memset(ones_row[:], 1.0)

            posT = cp.tile([P, NCOLS], f32)
            nc.sync.dma_start(out=posT[:], in_=posT_d[:])
            wT = []
            for h in range(HOPS):
                w = cp.tile([D, D], f32, tag=f"wt{h}", name=f"wt{h}")
                nc.sync.dma_start(out=w[:], in_=wT_d[h])
                wT.append(w)
            hop_bT = cp.tile([D, HOPS], f32)
            nc.sync.dma_start(out=hop_bT[:], in_=hop_bT_d[:])
            out_WT = cp.tile([D, S], f32)
            nc.sync.dma_start(out=out_WT[:], in_=out_WT_d[:])
            out_b = cp.tile([S, 1], f32)
            nc.sync.dma_start(out=out_b[:], in_=out_b_d[:])

            # PE warmups: observe every PE-read constant once so later
            # matmuls carry at most one fresh semaphore wait (walrus LDWEIGHTS
            # encodes a single wait slot).
            warm = psS.tile([P, P], f32, space="PSUM", tag="warm")
            nc.tensor.matmul(out=warm[:1, :1], lhsT=ident[:, :1], rhs=ident[:, :1],
                             start=True, stop=True)
            if EMB_BF16:
                nc.tensor.matmul(out=warm[:1, :1], lhsT=ident_e[:, :1],
                                 rhs=ident_e[:, :1], start=True, stop=True)
            nc.tensor.matmul(out=warm[:, :1], lhsT=ones_row[:], rhs=ones_row[:, :1],
                             start=True, stop=True)
            for h in range(HOPS):
                nc.tensor.matmul(out=warm[:1, :1], lhsT=wT[h][:, :1],
                                 rhs=wT[h][:, :1], start=True, stop=True)
            nc.tensor.matmul(out=warm[:1, :1], lhsT=out_WT[:, :1],
                             rhs=out_WT[:, :1], start=True, stop=True)

            gtok_reg = nc.gpsimd.to_reg(GTOK)

            # ---- question encode -> qT [D, bc] ----
            q0 = bp.tile([D, bc], f32)  # qT columns
            for g in range(nqblk):
                qidx = tkp.tile([P, GTOK // 16], i16, tag="idx")
                nc.sync.dma_start(out=qidx[:], in_=q_idx_d[g])
                qtok = tkp.tile([P, Q * D], EMB_DT, tag="tok")
                nc.gpsimd.dma_gather(
                    qtok[:].rearrange("p (j e) -> p j e", e=D),
                    emb_q[rebase:],
                    qidx[:],
                    GTOK,
                    gtok_reg,
                    D,
                    single_packet=False,
                )
                acc = psA.tile([P, D], f32, space="PSUM", tag="acc")
                for k in range(Q):
                    nc.tensor.matmul(
                        out=acc[:],
                        lhsT=ident_e[:],
                        rhs=qtok[:, k * D : (k + 1) * D],
                        start=(k == 0),
                        stop=(k == Q - 1),
                    )
                st = stp.tile([P, D], f32, tag="stage")
                nc.scalar.copy(out=st[:], in_=acc[:])
                tr = psB.tile([P, P], f32, space="PSUM", tag="tr")
                nc.tensor.transpose(out=tr[:], in_=st[:], identity=ident[:])
                nc.scalar.copy(out=q0[:, g * P : (g + 1) * P], in_=tr[:])

            # ---- ctx encode + hops, chunk-pipelined ----
            ctxT_chunks = [
                bp.tile([P, NCOLS], f32, tag=f"ctxT{c}", name=f"ctxT{c}")
                for c in range(nchunk)
            ]

            if debug:
                nc.sync.dma_start(out=dbg_q0_d[:], in_=q0[:])

            def emit_hops(c: int):
                if debug and c == 0:
                    nc.sync.dma_start(out=dbg_ctxT0_d[:], in_=ctxT_chunks[0][:])
                colbase = 0  # within chunk tile
                q_cur = q0[:, c * CHUNK_B : (c + 1) * CHUNK_B]
                ctxT = ctxT_chunks[c]
                for h in range(HOPS):
                    # absorb the fresh q_cur wait into a throwaway matmul
                    warm_a = psS.tile([P, P], f32, space="PSUM", tag="warm")
                    nc.tensor.matmul(out=warm_a[:1, :1], lhsT=q_cur[:, :1],
                                     rhs=q_cur[:, :1], start=True, stop=True)
                    # pqT = W_h @ q (+ bias)
                    pq_ps = psW.tile([P, 512], f32, space="PSUM", tag="wide")
                    nc.tensor.matmul(
                        out=pq_ps[:, :CHUNK_B],
                        lhsT=wT[h][:],
                        rhs=q_cur,
                        start=True,
                        stop=True,
                    )
                    pq = hp.tile([P, CHUNK_B], f32, tag="pq")
                    nc.vector.tensor_tensor(
                        out=pq[:],
                        in0=pq_ps[:, :CHUNK_B],
                        in1=hop_bT[:, h : h + 1].to_broadcast([P, CHUNK_B]),
                        op=mybir.AluOpType.add,
                    )
                    # scores: tmp = ctxT * repeat(pq, S)
                    tmp = hp.tile([P, NCOLS], f32, tag="tmp")
                    nc.vector.tensor_tensor(
                        out=tmp[:].rearrange("p (b s) -> p b s", s=S),
                        in0=ctxT[:].rearrange("p (b s) -> p b s", s=S),
                        in1=pq[:].to_broadcast([P, CHUNK_B, S]),
                        op=mybir.AluOpType.mult,
                    )
                    scores_flat = hp.tile([1, NCOLS], f32, tag="flat")
                    warm_b = psS.tile([P, P], f32, space="PSUM", tag="warm")
                    nc.tensor.matmul(out=warm_b[:1, :1], lhsT=tmp[:, :1],
                                     rhs=tmp[:, :1], start=True, stop=True)
                    for st0 in range(0, NCOLS, 512):
                        en = min(st0 + 512, NCOLS)
                        red = psW.tile([P, 512], f32, space="PSUM", tag="wide")
                        nc.tensor.matmul(
                            out=red[:1, : en - st0],
                            lhsT=ones_col[:],
                            rhs=tmp[:, st0:en],
                            start=True,
                            stop=True,
                        )
                        nc.scalar.copy(
                            out=scores_flat[:, st0:en], in_=red[:1, : en - st0]
                        )
                    sc2d = hp.tile([CHUNK_B, S], f32, tag="sc2d")
                    nc.sync.dma_start(out=sc2d[:], in_=scores_flat[:])
                    # softmax over free dim
                    negmax = hp.tile([CHUNK_B, 1], f32, tag="negmax")
                    nc.vector.tensor_reduce(
                        out=negmax[:],
                        in_=sc2d[:],
                        axis=mybir.AxisListType.X,
                        op=mybir.AluOpType.max,
                        negate=True,
                    )
                    expt = hp.tile([CHUNK_B, S], f32, tag="expt")
                    sumexp = hp.tile([CHUNK_B, 1], f32, tag="sumexp")
                    nc.scalar.activation(
                        out=expt[:],
                        in_=sc2d[:],
                        func=mybir.ActivationFunctionType.Exp,
                        bias=negmax[:],
                        accum_out=sumexp[:],
                    )
                    rcp = hp.tile([CHUNK_B, 1], f32, tag="rcp")
                    nc.vector.reciprocal(out=rcp[:], in_=sumexp[:])
                    attn2d = hp.tile([CHUNK_B, S], f32, tag="attn2d")
                    nc.vector.tensor_tensor(
                        out=attn2d[:],
                        in0=expt[:],
                        in1=rcp[:].to_broadcast([CHUNK_B, S]),
                        op=mybir.AluOpType.mult,
                    )
                    # write attn output
                    nc.sync.dma_start(
                        out=attn_d[h, c * CHUNK_B : (c + 1) * CHUNK_B, :],
                        in_=attn2d[:],
                    )
                    # retrieved: broadcast attn along partitions, mul, seg-reduce
                    attn_flat = hp.tile([1, NCOLS], f32, tag="flat")
                    nc.sync.dma_start(out=attn_flat[:], in_=attn2d[:])
                    attn_exp = hp.tile([P, NCOLS], f32, tag="attne", bufs=1)
                    warm_c = psS.tile([P, P], f32, space="PSUM", tag="warm")
                    nc.tensor.matmul(out=warm_c[:1, :1], lhsT=attn_flat[:, :1],
                                     rhs=attn_flat[:, :1], start=True, stop=True)
                    for st0 in range(0, NCOLS, 512):
                        en = min(st0 + 512, NCOLS)
                        bc_ps = psW.tile([P, 512], f32, space="PSUM", tag="wide")
                        nc.tensor.matmul(
                            out=bc_ps[:, : en - st0],
                            lhsT=ones_row[:],
                            rhs=attn_flat[:, st0:en],
                            start=True,
                            stop=True,
                        )
                        nc.scalar.copy(out=attn_exp[:, st0:en], in_=bc_ps[:, : en - st0])
                    tmp2 = hp.tile([P, NCOLS], f32, tag="tmp")
                    nc.vector.tensor_tensor(
                        out=tmp2[:], in0=ctxT[:], in1=attn_exp[:], op=mybir.AluOpType.mult
                    )
                    ret = hp.tile([P, CHUNK_B], f32, tag="ret")
                    nc.vector.tensor_reduce(
                        out=ret[:],
                        in_=tmp2[:].rearrange("p (b s) -> p b s", s=S),
                        axis=mybir.AxisListType.X,
                        op=mybir.AluOpType.add,
                    )
                    q_next = qp.tile([P, CHUNK_B], f32, tag="qn")
                    nc.vector.tensor_tensor(
                        out=q_next[:], in0=q_cur, in1=ret[:], op=mybir.AluOpType.add
                    )
                    q_cur = q_next[:]
                # logits for this chunk
                warm_d = psS.tile([P, P], f32, space="PSUM", tag="warm")
                nc.tensor.matmul(out=warm_d[:1, :1], lhsT=q_cur[:, :1],
                                 rhs=q_cur[:, :1], start=True, stop=True)
                lg_ps = psW.tile([P, 512], f32, space="PSUM", tag="wide")
                nc.tensor.matmul(
                    out=lg_ps[:S, :CHUNK_B],
                    lhsT=out_WT[:],
                    rhs=q_cur,
                    start=True,
                    stop=True,
                )
                lg = hp.tile([S, CHUNK_B], f32, tag="lg")
                nc.vector.tensor_tensor(
                    out=lg[:],
                    in0=lg_ps[:S, :CHUNK_B],
                    in1=out_b[:].to_broadcast([S, CHUNK_B]),
                    op=mybir.AluOpType.add,
                )
                nc.sync.dma_start(
                    out=logitsT_d[:, c * CHUNK_B : (c + 1) * CHUNK_B], in_=lg[:]
                )

            for t in range(nblk):
                cidx = tkp.tile([P, GTOK // 16], i16, tag="idx")
                nc.sync.dma_start(out=cidx[:], in_=ctx_idx_d[t])
                ctok = tkp.tile([P, L * D], EMB_DT, tag="tok")
                nc.gpsimd.dma_gather(
                    ctok[:].rearrange("p (j e) -> p j e", e=D),
                    emb_c[rebase:],
                    cidx[:],
                    GTOK,
                    gtok_reg,
                    D,
                    single_packet=False,
                )
                acc = psA.tile([P, D], f32, space="PSUM", tag="acc")
                for k in range(L):
                    nc.tensor.matmul(
                        out=acc[:],
                        lhsT=ident_e[:],
                        rhs=ctok[:, k * D : (k + 1) * D],
                        start=(k == 0),
                        stop=(k == L - 1),
                    )
                st = stp.tile([P, D], f32, tag="stage")
                nc.scalar.copy(out=st[:], in_=acc[:])
                tr = psB.tile([P, P], f32, space="PSUM", tag="tr")
                nc.tensor.transpose(out=tr[:], in_=st[:], identity=ident[:])
                c = t // BLK_PER_CHUNK
                tt = t % BLK_PER_CHUNK
                nc.scalar.copy(
                    out=ctxT_chunks[c][:, tt * P : (tt + 1) * P], in_=tr[:]
                )
                if tt == BLK_PER_CHUNK - 1:
                    # add pos_enc for the whole chunk in one DVE pass
                    nc.vector.tensor_tensor(
                        out=ctxT_chunks[c][:],
                        in0=ctxT_chunks[c][:],
                        in1=posT[:],
                        op=mybir.AluOpType.add,
                    )
                    emit_hops(c)

    if split_waits:
        _split_multi_waits(nc)
    mybir.codegen_inst_isa_subclasses(nc)
    nc.finalize()
    return nc


def _wrap_idxs(tok_blocks, rebase):
    """tok_blocks [nblk, 128 sentences, L tokens] int -> wrapped int16 index
    tensor [nblk, 128, (128*L)//16] in dma_gather layout: gather element
    i -> dst partition i%128, free block i//128; index i lives at SBUF
    [i%16 (+16g replicas), i//16]."""
    nblk = tok_blocks.shape[0]
    n = tok_blocks.shape[1] * tok_blocks.shape[2]
    # Sort each sentence's tokens ascending (sum is order-invariant) so the
    # last gather element of a call is the block's largest id: the ucode
    # drops a trailing run of negative (rebased) indices as padding.
    tok_blocks = np.sort(tok_blocks, axis=2)
    # element i = slot k*128 + p  ->  token k of sentence p
    flat = tok_blocks.transpose(0, 2, 1).reshape(nblk, n)
    v16 = (flat.astype(np.int64) - rebase).astype(np.int16)
    w16 = v16.reshape(nblk, n // 16, 16).swapaxes(1, 2)  # [nblk, 16, n//16]
    return np.ascontiguousarray(np.tile(w16, (1, 8, 1)))


def _prep_core_inputs(context_c, question_c, input_emb, question_emb, pos_enc,
                      hop_W, hop_b, out_W, out_b, bc, rebase=REBASE):
    nblk = (bc * S) // P
    nqblk = bc // P
    cf = context_c.reshape(bc * S, L)
    ctx_idx = _wrap_idxs(cf.reshape(nblk, P, L), rebase)
    qf = question_c.reshape(bc, Q)
    q_idx = _wrap_idxs(qf.reshape(nqblk, P, Q), rebase)
    posT_rep = np.tile(pos_enc.T, (1, NCOLS // S))
    if EMB_BF16:
        import ml_dtypes
        input_emb = input_emb.astype(ml_dtypes.bfloat16)
        question_emb = question_emb.astype(ml_dtypes.bfloat16)
    return {
        "emb_c": input_emb,
        "emb_q": question_emb,
        "ctx_idx": ctx_idx,
        "q_idx": q_idx,
        "posT_rep": np.ascontiguousarray(posT_rep, dtype=np.float32),
        "wT": np.ascontiguousarray(hop_W.transpose(0, 2, 1), dtype=np.float32),
        "hop_bT": np.ascontiguousarray(hop_b.T, dtype=np.float32),
        "out_WT": np.ascontiguousarray(out_W.T, dtype=np.float32),
        "out_b_col": np.ascontiguousarray(out_b[:, None], dtype=np.float32),
    }


def kernel(context, question, input_emb, question_emb, pos_enc, hop_W, hop_b,
           out_W, out_b):
    context = np.asarray(context).astype(np.int32)
    question = np.asarray(question).astype(np.int32)
    input_emb = np.ascontiguousarray(np.asarray(input_emb), dtype=np.float32)
    question_emb = np.ascontiguousarray(np.asarray(question_emb), dtype=np.float32)
    pos_enc = np.asarray(pos_enc, dtype=np.float32)
    hop_W = np.asarray(hop_W, dtype=np.float32)
    hop_b = np.asarray(hop_b, dtype=np.float32)
    out_W = np.asarray(out_W, dtype=np.float32)
    out_b = np.asarray(out_b, dtype=np.float32)

    nc = build_nc(BC)
    in_maps = []
    for c in range(NCORES):
        sl = slice(c * BC, (c + 1) * BC)
        in_maps.append(
            _prep_core_inputs(
                context[sl], question[sl], input_emb, question_emb, pos_enc,
                hop_W, hop_b, out_W, out_b, BC,
            )
        )

    trace = bool(int(os.environ.get("KERNEL_TRACE", "0")))
    res = run_bass_kernel_spmd(nc, in_maps, list(range(NCORES)), trace=trace)
    if trace and res.exec_time_ns is not None:
        print(f"HW exec time: {res.exec_time_ns} ns")

    logits = np.concatenate([r["logitsT"].T for r in res.results], axis=0)
    attn = np.concatenate([r["attn_out"] for r in res.results], axis=1)
    return logits.astype(np.float32), attn.astype(np.float32)


# revision 25
# speedup vs baseline: 75.9399x; 75.9399x over previous
"""End-to-end memory network kernel for Trainium2 (8 NeuronCores, batch-sharded).

Per core (BC = B/8 = 256 batch items):
  1. indirect-DMA gather of embedding rows (the memory-roofline term).
     Host pre-arranges token indices in partition-major blocks so each
     gather call delivers [128 sentences x 32 tokens x 128 dim].
  2. TensorE identity-matmul accumulation reduces the 32 tokens of each
     sentence into a PSUM tile  ctx[sigma, D]  (sigma = b*50 + s).
  3. PE transpose -> ctxT [D, sigma], fused pos_enc add on the way to SBUF.
  4. 3 memory hops on ctxT in transposed layouts:
       pqT = W_h @ qT (+b) ;  scores via DVE mul + ones-matmul partition
       reduce ;  softmax on [b, S] tiles ;  retrieved via attn broadcast
       (ones-matmul) + DVE mul + segmented reduce ;  qT += retrievedT.
  5. logitsT = out_W @ qT + out_b.
Outputs logitsT [50, BC] and attn [3, BC, 50] per core; host reassembles.
"""

import os

import numpy as np

import concourse.bass as bass
import concourse.mybir as mybir
import concourse.tile as tile
from concourse import library_config
from concourse.bass_utils import run_bass_kernel_spmd
from concourse.masks import make_identity

# Problem dims (hardcoded per harness contract)
B, S, L = 2048, 50, 32
Q = 32
D = 128
HOPS = 3
VOCAB = 50000
NCORES = 8
BC = B // NCORES  # 256

P = 128
REBASE = 25000  # dma_gather index rebase: idx16 = v - REBASE, base = emb[REBASE:]
GTOK = int(os.environ.get("KERNEL_GTOK", "4096"))  # tokens per dma_gather call
DMA_SCRATCH = int(os.environ.get("KERNEL_DMASCRATCH", "16384"))
CHUNK_B = 64  # batch items per hop chunk; CHUNK_B*S = 3200 = 25 blocks of 128
BLK_PER_CHUNK = (CHUNK_B * S) // P  # 25
NCOLS = CHUNK_B * S  # 3200

f32 = mybir.dt.float32
i32 = mybir.dt.int32
bf16 = mybir.dt.bfloat16
EMB_BF16 = bool(int(os.environ.get("KERNEL_BF16", "0")))
EMB_DT = bf16 if EMB_BF16 else f32


def _split_multi_waits(nc: bass.Bass) -> None:
    """This walrus build encodes a single sem-wait slot on several ISA
    structs (LDWEIGHTS, pseudo-DMA, ...). Rewrite every instruction that
    carries >1 wait: move all but the last wait onto NoOps inserted just
    before it on the same engine stream."""
    for bb in nc.bb_map.values():
        insts = bb.bb.instructions
        out = []
        changed = False
        for inst in insts:
            si = inst.sync_info
            waits = list(si.on_wait) if si is not None and si.on_wait else []
            if len(waits) > 1:
                for w in waits[:-1]:
                    out.append(
                        mybir.InstNoOp(
                            name=nc.get_next_instruction_name(),
                            engine=inst.engine,
                            ins=[],
                            outs=[],
                            sync_info=mybir.SyncInfo(on_wait=[w], on_update=[]),
                        )
                    )
                si.on_wait = [waits[-1]]
                changed = True
            out.append(inst)
        if changed:
            insts[:] = out


def build_nc(bc: int = BC, split_waits: bool = True, rebase: int = REBASE) -> bass.Bass:
    nsig = bc * S
    nblk = nsig // P
    nchunk = bc // CHUNK_B
    nqblk = (bc * Q) // (P * Q // Q)  # bc questions / 128 per block
    nqblk = bc // P  # blocks of 128 questions
    assert nsig % P == 0 and bc % CHUNK_B == 0 and bc % P == 0

    nc = bass.Bass(dynamic_dma_scratch_size=DMA_SCRATCH)

    emb_c = nc.dram_tensor("emb_c", [VOCAB, D], EMB_DT, kind="ExternalInput")
    emb_q = nc.dram_tensor("emb_q", [VOCAB, D], EMB_DT, kind="ExternalInput")
    i16 = mybir.dt.int16
    kslot = GTOK // P
    ctx_idx_d = nc.dram_tensor(
        "ctx_idx", [nblk * (L // kslot), P, GTOK // 16], i16, kind="ExternalInput"
    )
    q_idx_d = nc.dram_tensor(
        "q_idx", [nqblk * (Q // kslot), P, GTOK // 16], i16, kind="ExternalInput"
    )
    posT_d = nc.dram_tensor("posT_rep", [P, NCOLS], f32, kind="ExternalInput")
    wT_d = nc.dram_tensor("wT", [HOPS, D, D], f32, kind="ExternalInput")
    hop_bT_d = nc.dram_tensor("hop_bT", [D, HOPS], f32, kind="ExternalInput")
    out_WT_d = nc.dram_tensor("out_WT", [D, S], f32, kind="ExternalInput")
    out_b_d = nc.dram_tensor("out_b_col", [S, 1], f32, kind="ExternalInput")

    debug = bool(int(os.environ.get("KERNEL_DEBUG", "0")))
    if debug:
        dbg_q0_d = nc.dram_tensor("debug_q0", [D, bc], f32, kind="ExternalOutput")
        dbg_ctxT0_d = nc.dram_tensor(
            "debug_ctxT0", [P, NCOLS], f32, kind="ExternalOutput"
        )
    logitsT_d = nc.dram_tensor("logitsT", [S, bc], f32, kind="ExternalOutput")
    attn_d = nc.dram_tensor("attn_out", [HOPS, bc, S], f32, kind="ExternalOutput")

    nc.gpsimd.load_library(library_config.mlp)

    with tile.TileContext(nc) as tc:
        with (
            tc.tile_pool(name="const", bufs=1) as cp,
            tc.tile_pool(name="big", bufs=1) as bp,
            tc.tile_pool(name="tok", bufs=2) as tkp,
            tc.tile_pool(name="stage", bufs=3) as stp,
            tc.tile_pool(name="hop", bufs=2) as hp,
            tc.tile_pool(name="q", bufs=2 * HOPS + 2) as qp,
            tc.tile_pool(name="psA", bufs=2, space="PSUM") as psA,
            tc.tile_pool(name="psB", bufs=2, space="PSUM") as psB,
            tc.tile_pool(name="psW", bufs=3, space="PSUM") as psW,
            tc.tile_pool(name="psS", bufs=1, space="PSUM") as psS,
        ):
            # ---- constants / params ----
            ident = cp.tile([P, P], f32)
            make_identity(nc, ident[:])
            if EMB_BF16:
                ident_e = cp.tile([P, P], bf16, name="ident_e")
                nc.vector.tensor_copy(out=ident_e[:], in_=ident[:])
            else:
                ident_e = ident
            ones_col = cp.tile([P, 1], f32)
            nc.vector.memset(ones_col[:], 1.0)
            ones_row = cp.tile([1, P], f32)
            nc.vector.memset(ones_row[:], 1.0)

            posT = cp.tile([P, NCOLS], f32)
            nc.sync.dma_start(out=posT[:], in_=posT_d[:])
            wT = []
            for h in range(HOPS):
                w = cp.tile([D, D], f32, tag=f"wt{h}", name=f"wt{h}")
                nc.sync.dma_start(out=w[:], in_=wT_d[h])
                wT.append(w)
            hop_bT = cp.tile([D, HOPS], f32)
            nc.sync.dma_start(out=hop_bT[:], in_=hop_bT_d[:])
            out_WT = cp.tile([D, S], f32)
            nc.sync.dma_start(out=out_WT[:], in_=out_WT_d[:])
            out_b = cp.tile([S, 1], f32)
            nc.sync.dma_start(out=out_b[:], in_=out_b_d[:])

            # PE warmups: observe every PE-read constant once so later
            # matmuls carry at most one fresh semaphore wait (walrus LDWEIGHTS
            # encodes a single wait slot).
            warm = psS.tile([P, P], f32, space="PSUM", tag="warm")
            nc.tensor.matmul(out=warm[:1, :1], lhsT=ident[:, :1], rhs=ident[:, :1],
                             start=True, stop=True)
            if EMB_BF16:
                nc.tensor.matmul(out=warm[:1, :1], lhsT=ident_e[:, :1],
                                 rhs=ident_e[:, :1], start=True, stop=True)
            nc.tensor.matmul(out=warm[:, :1], lhsT=ones_row[:], rhs=ones_row[:, :1],
                             start=True, stop=True)
            for h in range(HOPS):
                nc.tensor.matmul(out=warm[:1, :1], lhsT=wT[h][:, :1],
                                 rhs=wT[h][:, :1], start=True, stop=True)
            nc.tensor.matmul(out=warm[:1, :1], lhsT=out_WT[:, :1],
                             rhs=out_WT[:, :1], start=True, stop=True)

            gtok_reg = nc.gpsimd.to_reg(GTOK)

            # ---- question encode -> qT [D, bc] ----
            q0 = bp.tile([D, bc], f32)  # qT columns
            for g in range(nqblk):
                qtok = tkp.tile([P, Q * D], EMB_DT, tag="tok")
                for s2 in range(Q // kslot):
                    qidx = tkp.tile([P, GTOK // 16], i16, tag="idx")
                    nc.sync.dma_start(
                        out=qidx[:], in_=q_idx_d[g * (Q // kslot) + s2]
                    )
                    nc.gpsimd.dma_gather(
                        qtok[:, s2 * kslot * D : (s2 + 1) * kslot * D].rearrange(
                            "p (j e) -> p j e", e=D
                        ),
                        emb_q[rebase:],
                        qidx[:],
                        GTOK,
                        gtok_reg,
                        D,
                        single_packet=False,
                    )
                acc = psA.tile([P, D], f32, space="PSUM", tag="acc")
                for k in range(Q):
                    nc.tensor.matmul(
                        out=acc[:],
                        lhsT=ident_e[:],
                        rhs=qtok[:, k * D : (k + 1) * D],
                        start=(k == 0),
                        stop=(k == Q - 1),
                    )
                st = stp.tile([P, D], f32, tag="stage")
                nc.scalar.copy(out=st[:], in_=acc[:])
                tr = psB.tile([P, P], f32, space="PSUM", tag="tr")
                nc.tensor.transpose(out=tr[:], in_=st[:], identity=ident[:])
                nc.scalar.copy(out=q0[:, g * P : (g + 1) * P], in_=tr[:])

            # ---- ctx encode + hops, chunk-pipelined ----
            ctxT_chunks = [
                bp.tile([P, NCOLS], f32, tag=f"ctxT{c}", name=f"ctxT{c}")
                for c in range(nchunk)
            ]

            if debug:
                nc.sync.dma_start(out=dbg_q0_d[:], in_=q0[:])

            qcur_map = {}

            def emit_hop(c: int, h: int):
                if debug and c == 0 and h == 0:
                    nc.sync.dma_start(out=dbg_ctxT0_d[:], in_=ctxT_chunks[0][:])
                q_cur = qcur_map[c]
                ctxT = ctxT_chunks[c]
                if True:
                    # absorb the fresh q_cur wait into a throwaway matmul
                    warm_a = psS.tile([P, P], f32, space="PSUM", tag="warm")
                    nc.tensor.matmul(out=warm_a[:1, :1], lhsT=q_cur[:, :1],
                                     rhs=q_cur[:, :1], start=True, stop=True)
                    # pqT = W_h @ q (+ bias)
                    pq_ps = psW.tile([P, 512], f32, space="PSUM", tag="wide")
                    nc.tensor.matmul(
                        out=pq_ps[:, :CHUNK_B],
                        lhsT=wT[h][:],
                        rhs=q_cur,
                        start=True,
                        stop=True,
                    )
                    pq = hp.tile([P, CHUNK_B], f32, tag="pq")
                    nc.vector.tensor_tensor(
                        out=pq[:],
                        in0=pq_ps[:, :CHUNK_B],
                        in1=hop_bT[:, h : h + 1].to_broadcast([P, CHUNK_B]),
                        op=mybir.AluOpType.add,
                    )
                    # scores: tmp = ctxT * repeat(pq, S)
                    tmp = hp.tile([P, NCOLS], f32, tag="tmp")
                    nc.vector.tensor_tensor(
                        out=tmp[:].rearrange("p (b s) -> p b s", s=S),
                        in0=ctxT[:].rearrange("p (b s) -> p b s", s=S),
                        in1=pq[:].to_broadcast([P, CHUNK_B, S]),
                        op=mybir.AluOpType.mult,
                    )
                    scores_flat = hp.tile([1, NCOLS], f32, tag="flat")
                    warm_b = psS.tile([P, P], f32, space="PSUM", tag="warm")
                    nc.tensor.matmul(out=warm_b[:1, :1], lhsT=tmp[:, :1],
                                     rhs=tmp[:, :1], start=True, stop=True)
                    for st0 in range(0, NCOLS, 512):
                        en = min(st0 + 512, NCOLS)
                        red = psW.tile([P, 512], f32, space="PSUM", tag="wide")
                        nc.tensor.matmul(
                            out=red[:1, : en - st0],
                            lhsT=ones_col[:],
                            rhs=tmp[:, st0:en],
                            start=True,
                            stop=True,
                        )
                        nc.scalar.copy(
                            out=scores_flat[:, st0:en], in_=red[:1, : en - st0]
                        )
                    sc2d = hp.tile([CHUNK_B, S], f32, tag="sc2d")
                    nc.sync.dma_start(out=sc2d[:], in_=scores_flat[:])
                    # softmax over free dim
                    negmax = hp.tile([CHUNK_B, 1], f32, tag="negmax")
                    nc.vector.tensor_reduce(
                        out=negmax[:],
                        in_=sc2d[:],
                        axis=mybir.AxisListType.X,
                        op=mybir.AluOpType.max,
                        negate=True,
                    )
                    expt = hp.tile([CHUNK_B, S], f32, tag="expt")
                    sumexp = hp.tile([CHUNK_B, 1], f32, tag="sumexp")
                    nc.scalar.activation(
                        out=expt[:],
                        in_=sc2d[:],
                        func=mybir.ActivationFunctionType.Exp,
                        bias=negmax[:],
                        accum_out=sumexp[:],
                    )
                    rcp = hp.tile([CHUNK_B, 1], f32, tag="rcp")
                    nc.vector.reciprocal(out=rcp[:], in_=sumexp[:])
                    attn2d = hp.tile([CHUNK_B, S], f32, tag="attn2d")
                    nc.vector.tensor_tensor(
                        out=attn2d[:],
                        in0=expt[:],
                        in1=rcp[:].to_broadcast([CHUNK_B, S]),
                        op=mybir.AluOpType.mult,
                    )
                    # write attn output
                    nc.sync.dma_start(
                        out=attn_d[h, c * CHUNK_B : (c + 1) * CHUNK_B, :],
                        in_=attn2d[:],
                    )
                    # retrieved: broadcast attn along partitions, mul, seg-reduce
                    attn_flat = hp.tile([1, NCOLS], f32, tag="flat")
                    nc.sync.dma_start(out=attn_flat[:], in_=attn2d[:])
                    attn_exp = hp.tile([P, NCOLS], f32, tag="attne", bufs=1)
                    warm_c = psS.tile([P, P], f32, space="PSUM", tag="warm")
                    nc.tensor.matmul(out=warm_c[:1, :1], lhsT=attn_flat[:, :1],
                                     rhs=attn_flat[:, :1], start=True, stop=True)
                    for st0 in range(0, NCOLS, 512):
                        en = min(st0 + 512, NCOLS)
                        bc_ps = psW.tile([P, 512], f32, space="PSUM", tag="wide")
                        nc.tensor.matmul(
                            out=bc_ps[:, : en - st0],
                            lhsT=ones_row[:],
                            rhs=attn_flat[:, st0:en],
                            start=True,
                            stop=True,
                        )
                        nc.scalar.copy(out=attn_exp[:, st0:en], in_=bc_ps[:, : en - st0])
                    tmp2 = hp.tile([P, NCOLS], f32, tag="tmp")
                    nc.vector.tensor_tensor(
                        out=tmp2[:], in0=ctxT[:], in1=attn_exp[:], op=mybir.AluOpType.mult
                    )
                    ret = hp.tile([P, CHUNK_B], f32, tag="ret")
                    nc.vector.tensor_reduce(
                        out=ret[:],
                        in_=tmp2[:].rearrange("p (b s) -> p b s", s=S),
                        axis=mybir.AxisListType.X,
                        op=mybir.AluOpType.add,
                    )
                    q_next = qp.tile([P, CHUNK_B], f32, tag="qn")
                    nc.vector.tensor_tensor(
                        out=q_next[:], in0=q_cur, in1=ret[:], op=mybir.AluOpType.add
                    )
                    qcur_map[c] = q_next[:]

            def emit_logits(c: int):
                q_cur = qcur_map[c]
                warm_d = psS.tile([P, P], f32, space="PSUM", tag="warm")
                nc.tensor.matmul(out=warm_d[:1, :1], lhsT=q_cur[:, :1],
                                 rhs=q_cur[:, :1], start=True, stop=True)
                lg_ps = psW.tile([P, 512], f32, space="PSUM", tag="wide")
                nc.tensor.matmul(
                    out=lg_ps[:S, :CHUNK_B],
                    lhsT=out_WT[:],
                    rhs=q_cur,
                    start=True,
                    stop=True,
                )
                lg = hp.tile([S, CHUNK_B], f32, tag="lg")
                nc.vector.tensor_tensor(
                    out=lg[:],
                    in0=lg_ps[:S, :CHUNK_B],
                    in1=out_b[:].to_broadcast([S, CHUNK_B]),
                    op=mybir.AluOpType.add,
                )
                nc.sync.dma_start(
                    out=logitsT_d[:, c * CHUNK_B : (c + 1) * CHUNK_B], in_=lg[:]
                )

            nohops = bool(int(os.environ.get("KERNEL_NOHOPS", "0")))
            pending = []
            for t in range(nblk):
                ctok = tkp.tile([P, L * D], EMB_DT, tag="tok")
                for s2 in range(L // kslot):
                    cidx = tkp.tile([P, GTOK // 16], i16, tag="idx")
                    nc.sync.dma_start(
                        out=cidx[:], in_=ctx_idx_d[t * (L // kslot) + s2]
                    )
                    nc.gpsimd.dma_gather(
                        ctok[:, s2 * kslot * D : (s2 + 1) * kslot * D].rearrange(
                            "p (j e) -> p j e", e=D
                        ),
                        emb_c[rebase:],
                        cidx[:],
                        GTOK,
                        gtok_reg,
                        D,
                        single_packet=False,
                    )
                acc = psA.tile([P, D], f32, space="PSUM", tag="acc")
                for k in range(L):
                    nc.tensor.matmul(
                        out=acc[:],
                        lhsT=ident_e[:],
                        rhs=ctok[:, k * D : (k + 1) * D],
                        start=(k == 0),
                        stop=(k == L - 1),
                    )
                st = stp.tile([P, D], f32, tag="stage")
                nc.scalar.copy(out=st[:], in_=acc[:])
                tr = psB.tile([P, P], f32, space="PSUM", tag="tr")
                nc.tensor.transpose(out=tr[:], in_=st[:], identity=ident[:])
                c = t // BLK_PER_CHUNK
                tt = t % BLK_PER_CHUNK
                nc.scalar.copy(
                    out=ctxT_chunks[c][:, tt * P : (tt + 1) * P], in_=tr[:]
                )
                if tt == BLK_PER_CHUNK - 1:
                    # add pos_enc for the whole chunk in one DVE pass
                    nc.vector.tensor_tensor(
                        out=ctxT_chunks[c][:],
                        in0=ctxT_chunks[c][:],
                        in1=posT[:],
                        op=mybir.AluOpType.add,
                    )
                    if not nohops:
                        qcur_map[c] = q0[:, c * CHUNK_B : (c + 1) * CHUNK_B]
                        for h in range(HOPS):
                            pending.append((emit_hop, (c, h)))
                        pending.append((emit_logits, (c,)))
                elif tt % 6 == 5 and pending:
                    fn, args = pending.pop(0)
                    fn(*args)

            while pending:
                fn, args = pending.pop(0)
                fn(*args)

    if split_waits:
        _split_multi_waits(nc)
    mybir.codegen_inst_isa_subclasses(nc)
    nc.finalize()
    return nc


def _wrap_idxs(tok_blocks, rebase):
    """tok_blocks [nblk, 128 sentences, L tokens] int -> wrapped int16 index
    tensor [nblk, 128, (128*L)//16] in dma_gather layout: gather element
    i -> dst partition i%128, free block i//128; index i lives at SBUF
    [i%16 (+16g replicas), i//16]."""
    nblk = tok_blocks.shape[0]
    n = tok_blocks.shape[1] * tok_blocks.shape[2]
    # Sort each sentence's tokens ascending (sum is order-invariant) so the
    # last gather element of a call is the block's largest id: the ucode
    # drops a trailing run of negative (rebased) indices as padding.
    tok_blocks = np.sort(tok_blocks, axis=2)
    # element i = slot k*128 + p  ->  token k of sentence p
    flat = tok_blocks.transpose(0, 2, 1).reshape(nblk, n)
    v16 = (flat.astype(np.int64) - rebase).astype(np.int16)
    w16 = v16.reshape(nblk, n // 16, 16).swapaxes(1, 2)  # [nblk, 16, n//16]
    return np.ascontiguousarray(np.tile(w16, (1, 8, 1)))


def _split_calls(blocks):
    """[nblk, 128, L] -> [nblk * L//KSLOT, 128, KSLOT] sub-call token lists."""
    nblk, p, l = blocks.shape
    ks = GTOK // P
    return (
        blocks.reshape(nblk, p, l // ks, ks)
        .transpose(0, 2, 1, 3)
        .reshape(nblk * (l // ks), p, ks)
    )


def _prep_core_inputs(context_c, question_c, input_emb, question_emb, pos_enc,
                      hop_W, hop_b, out_W, out_b, bc, rebase=REBASE):
    nblk = (bc * S) // P
    nqblk = bc // P
    cf = context_c.reshape(bc * S, L)
    ctx_idx = _wrap_idxs(_split_calls(cf.reshape(nblk, P, L)), rebase)
    qf = question_c.reshape(bc, Q)
    q_idx = _wrap_idxs(_split_calls(qf.reshape(nqblk, P, Q)), rebase)
    posT_rep = np.tile(pos_enc.T, (1, NCOLS // S))
    if EMB_BF16:
        import ml_dtypes
        input_emb = input_emb.astype(ml_dtypes.bfloat16)
        question_emb = question_emb.astype(ml_dtypes.bfloat16)
    return {
        "emb_c": input_emb,
        "emb_q": question_emb,
        "ctx_idx": ctx_idx,
        "q_idx": q_idx,
        "posT_rep": np.ascontiguousarray(posT_rep, dtype=np.float32),
        "wT": np.ascontiguousarray(hop_W.transpose(0, 2, 1), dtype=np.float32),
        "hop_bT": np.ascontiguousarray(hop_b.T, dtype=np.float32),
        "out_WT": np.ascontiguousarray(out_W.T, dtype=np.float32),
        "out_b_col": np.ascontiguousarray(out_b[:, None], dtype=np.float32),
    }


def kernel(context, question, input_emb, question_emb, pos_enc, hop_W, hop_b,
           out_W, out_b):
    context = np.asarray(context).astype(np.int32)
    question = np.asarray(question).astype(np.int32)
    input_emb = np.ascontiguousarray(np.asarray(input_emb), dtype=np.float32)
    question_emb = np.ascontiguousarray(np.asarray(question_emb), dtype=np.float32)
    pos_enc = np.asarray(pos_enc, dtype=np.float32)
    hop_W = np.asarray(hop_W, dtype=np.float32)
    hop_b = np.asarray(hop_b, dtype=np.float32)
    out_W = np.asarray(out_W, dtype=np.float32)
    out_b = np.asarray(out_b, dtype=np.float32)

    nc = build_nc(BC)
    in_maps = []
    for c in range(NCORES):
        sl = slice(c * BC, (c + 1) * BC)
        in_maps.append(
            _prep_core_inputs(
                context[sl], question[sl], input_emb, question_emb, pos_enc,
                hop_W, hop_b, out_W, out_b, BC,
            )
        )

    trace = bool(int(os.environ.get("KERNEL_TRACE", "0")))
    try:
        res = run_bass_kernel_spmd(nc, in_maps, list(range(NCORES)), trace=trace)
    except ModuleNotFoundError:
        # no NTFF profiling hook in this container; run without tracing
        res = run_bass_kernel_spmd(nc, in_maps, list(range(NCORES)), trace=False)
    if trace and res.exec_time_ns is not None:
        print(f"HW exec time: {res.exec_time_ns} ns")

    logits = np.concatenate([r["logitsT"].T for r in res.results], axis=0)
    attn = np.concatenate([r["attn_out"] for r in res.results], axis=1)
    return logits.astype(np.float32), attn.astype(np.float32)
